# revision 2
# baseline (speedup 1.0000x reference)
"""Trainium2 Bass kernel for GrapherModule — single-launch version.

fc1+BN1 -> exact KNN(k=9) -> MaxRelative conv+BN+GELU -> fc2+BN -> +residual.

Sharding: 8 cores; core d handles batch b=d//4, query slice qoff=(d%4)*2048.
x is np.roll'ed by -qoff per core so queries are local rows 0..2047 and the
SPMD program is identical on every core.

Design notes (vs the 2-launch baseline):
- Distance scores s[q,j] = h_q.h_j - 0.5|h_j|^2 computed with a bf16 hi/lo
  split (3 matmul passes, ~1e-4 abs error — top-8 matches fp32 selection)
  at 1 cycle/row with fast bf16 weight loads, vs fp32's 4 cycles/row + slow
  LDWEIGHTS. The -0.5|h_j|^2 bias is added inside the same PSUM accumulation
  by one extra 2-contraction-row matmul against a bf16 (hi,lo) bias pair,
  so score drains are plain Act copies.
- DVE does only max8 + max_index (exact top-8) + the 9-way neighbor max.
- Neighbor gather runs on-device: indirect DMA row-gather from a DRAM copy
  of pre-BN h (token-major; runs padded so each row is its own descriptor);
  BN1's scale is applied after the max (gamma>0 commutes), which lets the
  h_tok DMA overlap the BN1 stats AllReduce.
- BN stats (all three) via tiny [128,2] AllReduce collectives.
"""
import sys, os
sys.path.insert(0, '/opt/trn_rl_repo')
os.environ.setdefault('JAX_PLATFORMS', 'cpu')

import numpy as np

B, N, C = 2, 8192, 128
K = 9
NQ = 2048          # queries per core
NT = NQ // 128     # 16 query tiles per core
GP = 132           # padded per-neighbor stride in the gather tile
EPS = 1e-5

_CACHE = {}


def _build():
    import concourse.bass as bass
    import concourse.mybir as mybir
    import concourse.tile as tile
    from concourse import bacc
    from concourse.masks import make_identity

    dt = mybir.dt
    AF = mybir.ActivationFunctionType
    ALU = mybir.AluOpType
    AX = mybir.AxisListType

    nc = bacc.Bacc("TRN2", target_bir_lowering=False, debug=False,
                   enable_asserts=False, num_devices=8)

    # ---- I/O (weights pre-transposed on host) ----
    x_own = nc.dram_tensor("x_own", [N, C], dt.float32, kind="ExternalInput")
    fc1_wT = nc.dram_tensor("fc1_wT", [C, C], dt.float32, kind="ExternalInput")
    fc1_b = nc.dram_tensor("fc1_b", [C], dt.float32, kind="ExternalInput")
    bn1_g = nc.dram_tensor("bn1_g", [C], dt.float32, kind="ExternalInput")
    bn1_b = nc.dram_tensor("bn1_b", [C], dt.float32, kind="ExternalInput")
    v1_T = nc.dram_tensor("v1_T", [C, C], dt.float32, kind="ExternalInput")   # (cw1-cw2)^T
    cw2_T = nc.dram_tensor("cw2_T", [C, C], dt.float32, kind="ExternalInput")
    conv_b = nc.dram_tensor("conv_b", [C], dt.float32, kind="ExternalInput")
    bnc_g = nc.dram_tensor("bnc_g", [C], dt.float32, kind="ExternalInput")
    bnc_b = nc.dram_tensor("bnc_b", [C], dt.float32, kind="ExternalInput")
    fc2_wT = nc.dram_tensor("fc2_wT", [C, C], dt.float32, kind="ExternalInput")
    fc2_b = nc.dram_tensor("fc2_b", [C], dt.float32, kind="ExternalInput")
    bn2_g = nc.dram_tensor("bn2_g", [C], dt.float32, kind="ExternalInput")
    bn2_b = nc.dram_tensor("bn2_b", [C], dt.float32, kind="ExternalInput")
    y = nc.dram_tensor("y", [NQ, C], dt.float32, kind="ExternalOutput")
    DBG = os.environ.get('KDBG') == '1'
    if DBG:
        dbg_h = nc.dram_tensor("dbg_h", [128, N], dt.float32, kind="ExternalOutput")
        dbg_s = nc.dram_tensor("dbg_s", [128, N], dt.float32, kind="ExternalOutput")
        dbg_i = nc.dram_tensor("dbg_i", [128, 9], dt.uint32, kind="ExternalOutput")
        dbg_g9 = nc.dram_tensor("dbg_g9", [128, 9 * GP], dt.float32, kind="ExternalOutput")
        dbg_at = nc.dram_tensor("dbg_at", [128, NQ], dt.float32, kind="ExternalOutput")
        dbg_ht = nc.dram_tensor("dbg_ht", [N, C], dt.float32, kind="ExternalOutput")

    def col(t):
        return t[:].rearrange("(c one) -> c one", one=1)

    with tile.TileContext(nc) as tc:
        wpool = tc.alloc_tile_pool(name="w", bufs=1)
        pers = tc.alloc_tile_pool(name="pers", bufs=1)
        dram = tc.alloc_tile_pool(name="dram", bufs=2, space="DRAM")
        dramP = tc.alloc_tile_pool(name="dramP", bufs=1, space="DRAM")
        h_tok = dramP.tile([N, C], dt.float32)   # pre-BN h, token-major (gather source)

        ident = wpool.tile([128, 128], dt.float32)
        make_identity(nc, ident[:])
        ones128 = wpool.tile([128, 128], dt.float32)
        nc.vector.memset(ones128[:], 1.0)
        ones2 = wpool.tile([2, 128], dt.bfloat16)
        nc.vector.memset(ones2[:], 1.0)

        fc1wT = wpool.tile([C, C], dt.float32)
        nc.sync.dma_start(fc1wT[:], fc1_wT[:, :])
        v1T = wpool.tile([C, C], dt.float32)
        nc.sync.dma_start(v1T[:], v1_T[:, :])
        cw2T = wpool.tile([C, C], dt.float32)
        nc.sync.dma_start(cw2T[:], cw2_T[:, :])
        fc2wT = wpool.tile([C, C], dt.float32)
        nc.sync.dma_start(fc2wT[:], fc2_wT[:, :])
        fc1b = wpool.tile([C, 1], dt.float32); nc.sync.dma_start(fc1b[:], col(fc1_b))
        bn1g = wpool.tile([C, 1], dt.float32); nc.sync.dma_start(bn1g[:], col(bn1_g))
        bn1bb = wpool.tile([C, 1], dt.float32); nc.sync.dma_start(bn1bb[:], col(bn1_b))
        convb = wpool.tile([C, 1], dt.float32); nc.sync.dma_start(convb[:], col(conv_b))
        bncg = wpool.tile([C, 1], dt.float32); nc.sync.dma_start(bncg[:], col(bnc_g))
        bncb = wpool.tile([C, 1], dt.float32); nc.sync.dma_start(bncb[:], col(bnc_b))
        fc2b = wpool.tile([C, 1], dt.float32); nc.sync.dma_start(fc2b[:], col(fc2_b))
        bn2g = wpool.tile([C, 1], dt.float32); nc.sync.dma_start(bn2g[:], col(bn2_g))
        bn2bb = wpool.tile([C, 1], dt.float32); nc.sync.dma_start(bn2bb[:], col(bn2_b))

        # persistent SBUF
        h = pers.tile([128, N], dt.float32)       # feature-major h (pre-BN, then normalized)
        h_hi = pers.tile([128, N], dt.bfloat16)   # bf16 split of normalized h
        h_lo = pers.tile([128, N], dt.bfloat16)
        nbp = pers.tile([2, N], dt.bfloat16)      # bias pair: row0=hi, row1=lo of -0.5|h_j|^2
        x_resT = pers.tile([128, NQ], dt.float32)
        aggtok = pers.tile([128, NQ], dt.float32)
        sum_p = pers.tile([128, 4], dt.float32)
        ssq_p = pers.tile([128, 4], dt.float32)

        # ---------- Phase A: load x, transpose, fc1 (plain fp32) ----------
        with tc.tile_pool(name="phA", bufs=3) as phA, \
             tc.tile_pool(name="psA", bufs=2, space="PSUM") as psA, \
             tc.tile_pool(name="psF", bufs=2, space="PSUM") as psF:
            for blk in range(16):
                xTb = phA.tile([128, 512], dt.float32, tag="xT")
                for j in range(4):
                    t = blk * 4 + j
                    r0 = t * 128
                    xt = phA.tile([128, 128], dt.float32, tag="xt")
                    nc.sync.dma_start(xt[:], x_own[r0:r0 + 128, :])
                    pxt = psA.tile([128, 128], dt.float32, tag="pT")
                    nc.tensor.transpose(pxt[:], xt[:], ident[:])
                    nc.vector.tensor_copy(xTb[:, j * 128:(j + 1) * 128], pxt[:])
                    if blk < 4:
                        nc.vector.tensor_copy(
                            x_resT[:, blk * 512 + j * 128:blk * 512 + (j + 1) * 128],
                            pxt[:])
                pre = psF.tile([128, 512], dt.float32, tag="pF")
                nc.tensor.matmul(pre[:], fc1wT[:], xTb[:], start=True, stop=True)
                sl = slice(blk * 512, (blk + 1) * 512)
                if blk < 4:
                    nc.scalar.activation(h[:, sl], pre[:], AF.Identity,
                                         bias=fc1b[:], accum_out=sum_p[:, blk:blk + 1])
                else:
                    nc.scalar.activation(h[:, sl], pre[:], AF.Identity, bias=fc1b[:])

        # sum of squares over query shard
        with tc.tile_pool(name="sq", bufs=2) as sqp:
            for blkq in range(4):
                sl = slice(blkq * 512, (blkq + 1) * 512)
                jq = sqp.tile([128, 512], dt.float32, tag="jq")
                nc.scalar.activation(jq[:], h[:, sl], AF.Square,
                                     accum_out=ssq_p[:, blkq:blkq + 1])

        # ---------- h_tok (pre-BN, token-major) to DRAM; overlaps the AllReduce ----------
        with tc.tile_pool(name="ht", bufs=3) as htp, \
             tc.tile_pool(name="psT", bufs=2, space="PSUM") as psT:
            for g in range(16):
                pt = psT.tile([128, 512], dt.float32, tag="pT4")
                for j in range(4):
                    i = g * 4 + j
                    nc.tensor.transpose(pt[:, j * 128:(j + 1) * 128],
                                        h[:, i * 128:(i + 1) * 128], ident[:])
                st = htp.tile([128, 512], dt.float32, tag="st")
                nc.vector.tensor_copy(st[:], pt[:])
                nc.sync.dma_start(
                    h_tok[g * 512:(g + 1) * 512, :]
                        .rearrange("(four p) c -> p four c", four=4),
                    st[:])

        def allreduce2(sump, ssqp):
            loc = pers.tile([128, 2], dt.float32)
            nc.vector.reduce_sum(loc[:, 0:1], sump[:], axis=AX.X)
            nc.vector.reduce_sum(loc[:, 1:2], ssqp[:], axis=AX.X)
            bin_ = dram.tile([128, 2], dt.float32)
            bout = dram.tile([128, 2], dt.float32)
            nc.gpsimd.dma_start(bin_[:], loc[:])
            nc.gpsimd.collective_compute(
                "AllReduce", ALU.add, replica_groups=[list(range(8))],
                ins=[bin_.opt()], outs=[bout.opt()])
            tot = pers.tile([128, 2], dt.float32)
            nc.gpsimd.dma_start(tot[:], bout[:])
            return tot

        def bnparams(tot, gam, bet):
            st = pers.tile([128, 8], dt.float32)
            mm, e2, vv, rr, sc, bi = (st[:, i:i + 1] for i in range(6))
            nc.vector.tensor_scalar_mul(mm, tot[:, 0:1], 1.0 / (B * N))
            nc.vector.tensor_scalar_mul(e2, tot[:, 1:2], 1.0 / (B * N))
            nc.vector.tensor_tensor(vv, mm, mm, op=ALU.mult)
            nc.vector.tensor_sub(vv, e2, vv)
            nc.vector.tensor_scalar(vv, vv, EPS, None, op0=ALU.add)
            nc.vector.reciprocal(rr, vv)
            nc.scalar.activation(rr, rr, AF.Sqrt)
            nc.vector.tensor_tensor(sc, rr, gam, op=ALU.mult)
            nc.vector.tensor_tensor(bi, mm, sc, op=ALU.mult)
            nc.vector.tensor_sub(bi, bet, bi)
            return sc, bi

        sc1, bi1 = bnparams(allreduce2(sum_p, ssq_p), bn1g[:], bn1bb[:])
        # normalize h in place
        nc.scalar.activation(h[:], h[:], AF.Identity, bias=bi1, scale=sc1)
        if DBG:
            nc.sync.dma_start(dbg_h[:], h[:])
            nc.sync.dma_start(dbg_ht[:], h_tok[:])

        # ---------- bf16 hi/lo split of h + negx2 bias pair ----------
        with tc.tile_pool(name="sp", bufs=3) as spp, \
             tc.tile_pool(name="psN", bufs=2, space="PSUM") as psN:
            for cblk in range(16):
                sl = slice(cblk * 512, (cblk + 1) * 512)
                # h_hi = bf16(h); h_lo = bf16(h - fp32(h_hi))
                nc.scalar.activation(h_hi[:, sl], h[:, sl], AF.Identity)
                hi32 = spp.tile([128, 512], dt.float32, tag="hi32")
                nc.scalar.activation(hi32[:], h_hi[:, sl], AF.Identity)
                nc.vector.tensor_sub(h_lo[:, sl], h[:, sl], hi32[:])
                # negx2 = -0.5*colsum(h*h) via fp32 ones-matmul
                hh = spp.tile([128, 512], dt.float32, tag="hh")
                nc.vector.tensor_tensor(hh[:], h[:, sl], h[:, sl], op=ALU.mult)
                pn = psN.tile([128, 512], dt.float32, tag="pN")
                nc.tensor.matmul(pn[:], ones128[:], hh[:], start=True, stop=True)
                nb32 = spp.tile([1, 512], dt.float32, tag="nb32")
                nc.scalar.activation(nb32[:], pn[0:1, :], AF.Copy, scale=-0.5)
                nc.vector.tensor_copy(nbp[0:1, sl], nb32[:])
                nbh32 = spp.tile([1, 512], dt.float32, tag="nbh32")
                nc.scalar.activation(nbh32[:], nbp[0:1, sl], AF.Identity)
                nblo = spp.tile([1, 512], dt.bfloat16, tag="nblo")
                nc.vector.tensor_sub(nblo[:], nb32[:], nbh32[:])
                nc.sync.dma_start(nbp[1:2, sl], nblo[:])

        # ---------- Phase B: scores + exact top-8 + gather + neighbor max ----------
        with tc.tile_pool(name="srow", bufs=2) as srp, \
             tc.tile_pool(name="smal", bufs=3) as smal, \
             tc.tile_pool(name="gth", bufs=2) as gth, \
             tc.tile_pool(name="psB", bufs=2, space="PSUM") as psB:
            for i in range(NT):
                q0 = i * 128
                s = srp.tile([128, N], dt.float32, tag="s")
                for half in range(4):
                    ps = psB.tile([128, 2048], dt.float32, tag="pB")
                    for j in range(4):
                        c0 = half * 2048 + j * 512
                        psl = slice(j * 512, (j + 1) * 512)
                        cs = slice(c0, c0 + 512)
                        nc.tensor.matmul(ps[:, psl], ones2[:], nbp[:, cs],
                                         start=True, stop=False)
                        nc.tensor.matmul(ps[:, psl], h_hi[:, q0:q0 + 128],
                                         h_hi[:, cs], start=False, stop=False)
                        nc.tensor.matmul(ps[:, psl], h_hi[:, q0:q0 + 128],
                                         h_lo[:, cs], start=False, stop=False)
                        nc.tensor.matmul(ps[:, psl], h_lo[:, q0:q0 + 128],
                                         h_hi[:, cs], start=False, stop=True)
                    nc.scalar.activation(s[:, half * 2048:(half + 1) * 2048],
                                         ps[:], AF.Identity)
                # mask self
                nc.gpsimd.affine_select(
                    s[:, q0:q0 + 128], s[:, q0:q0 + 128],
                    pattern=[[1, 128]], compare_op=ALU.not_equal,
                    fill=-1e30, base=0, channel_multiplier=-1)
                top8v = smal.tile([128, 8], dt.float32, tag="t8v")
                nc.vector.max(top8v[:], s[:])
                idx9 = smal.tile([128, 9], dt.uint32, tag="i9")
                nc.gpsimd.iota(idx9[:, 0:1], pattern=[[0, 1]], base=q0,
                               channel_multiplier=1)
                nc.vector.max_index(idx9[:, 1:9], top8v[:], s[:])
                if DBG and i == 0:
                    nc.sync.dma_start(dbg_s[:], s[:])
                    nc.sync.dma_start(dbg_i[:], idx9[:])
                # gather 9 neighbor rows (pre-BN h_tok) per query: one
                # indirect DMA per neighbor slot ([128,1] offsets -> [128,128]
                # rows), the layout the DGE pairs per-partition.
                g9 = gth.tile([128, K, GP], dt.float32, tag="g9")
                for r in range(K):
                    nc.gpsimd.indirect_dma_start(
                        g9[:, r, 0:128], None, h_tok[:],
                        bass.IndirectOffsetOnAxis(ap=idx9[:, r:r + 1], axis=0))
                # neighbor max over 9 (DVE tensor_tensor max tree)
                def smax(dst, a, bsl):
                    nc.vector.tensor_tensor(dst, a, bsl, op=ALU.max)
                m0 = smal.tile([128, 128], dt.float32, tag="m0")
                m1 = smal.tile([128, 128], dt.float32, tag="m1")
                m2 = smal.tile([128, 128], dt.float32, tag="m2")
                m3 = smal.tile([128, 128], dt.float32, tag="m3")
                smax(m0[:], g9[:, 0, 0:128], g9[:, 1, 0:128])
                smax(m1[:], g9[:, 2, 0:128], g9[:, 3, 0:128])
                smax(m2[:], g9[:, 4, 0:128], g9[:, 5, 0:128])
                smax(m3[:], g9[:, 6, 0:128], g9[:, 7, 0:128])
                smax(m0[:], m0[:], m1[:])
                smax(m2[:], m2[:], m3[:])
                smax(m0[:], m0[:], m2[:])
                smax(aggtok[:, q0:q0 + 128], m0[:], g9[:, 8, 0:128])
                if DBG and i == 0:
                    nc.sync.dma_start(dbg_g9[:], g9[:])

        if DBG:
            nc.sync.dma_start(dbg_at[:], aggtok[:])

        # ---------- Phase C: conv + BNc + GELU, fc2 + BN2, out ----------
        convpre = pers.tile([128, NQ], dt.float32)
        agg_fm = pers.tile([128, NQ], dt.float32)
        csum_p = pers.tile([128, 4], dt.float32)
        cssq_p = pers.tile([128, 4], dt.float32)
        fsum_p = pers.tile([128, 4], dt.float32)
        fssq_p = pers.tile([128, 4], dt.float32)

        with tc.tile_pool(name="phC", bufs=2) as phC, \
             tc.tile_pool(name="psC", bufs=2, space="PSUM") as psC:
            # transpose aggtok -> agg_fm, then apply BN1 scale/bias (gamma>0)
            for g in range(4):
                pt = psC.tile([128, 512], dt.float32, tag="pCt")
                for j in range(4):
                    i = g * 4 + j
                    nc.tensor.transpose(pt[:, j * 128:(j + 1) * 128],
                                        aggtok[:, i * 128:(i + 1) * 128], ident[:])
                nc.vector.tensor_copy(agg_fm[:, g * 512:(g + 1) * 512], pt[:])
            nc.scalar.activation(agg_fm[:], agg_fm[:], AF.Identity,
                                 bias=bi1, scale=sc1)

            for cb in range(4):
                sl = slice(cb * 512, (cb + 1) * 512)
                pc = psC.tile([128, 512], dt.float32, tag="pCc")
                nc.tensor.matmul(pc[:], v1T[:], h[:, sl], start=True, stop=False)
                nc.tensor.matmul(pc[:], cw2T[:], agg_fm[:, sl],
                                 start=False, stop=True)
                nc.scalar.activation(convpre[:, sl], pc[:], AF.Identity,
                                     bias=convb[:], accum_out=csum_p[:, cb:cb + 1])
                jq = phC.tile([128, 512], dt.float32, tag="cj")
                nc.scalar.activation(jq[:], convpre[:, sl], AF.Square,
                                     accum_out=cssq_p[:, cb:cb + 1])

            scc, bic = bnparams(allreduce2(csum_p, cssq_p), bncg[:], bncb[:])
            g_act = convpre   # GELU in place
            nc.scalar.activation(g_act[:], convpre[:], AF.Gelu, bias=bic, scale=scc)

            f2pre = agg_fm    # reuse buffer
            for cb in range(4):
                sl = slice(cb * 512, (cb + 1) * 512)
                pf = psC.tile([128, 512], dt.float32, tag="pCf")
                nc.tensor.matmul(pf[:], fc2wT[:], g_act[:, sl],
                                 start=True, stop=True)
                nc.scalar.activation(f2pre[:, sl], pf[:], AF.Identity, bias=fc2b[:],
                                     accum_out=fsum_p[:, cb:cb + 1])
                jf = phC.tile([128, 512], dt.float32, tag="fj")
                nc.scalar.activation(jf[:], f2pre[:, sl], AF.Square,
                                     accum_out=fssq_p[:, cb:cb + 1])

            scf, bif = bnparams(allreduce2(fsum_p, fssq_p), bn2g[:], bn2bb[:])
            outfm = pers.tile([128, NQ], dt.float32)
            nc.scalar.activation(outfm[:], f2pre[:], AF.Identity,
                                 bias=bif, scale=scf)
            nc.vector.tensor_add(outfm[:], outfm[:], x_resT[:])

            for g in range(4):
                po = psC.tile([128, 512], dt.float32, tag="pCo")
                for j in range(4):
                    i = g * 4 + j
                    nc.tensor.transpose(po[:, j * 128:(j + 1) * 128],
                                        outfm[:, i * 128:(i + 1) * 128], ident[:])
                ot = phC.tile([128, 512], dt.float32, tag="ot")
                nc.vector.tensor_copy(ot[:], po[:])
                nc.sync.dma_start(
                    y[g * 512:(g + 1) * 512, :]
                        .rearrange("(four p) c -> p four c", four=4),
                    ot[:])

        for p in (dramP, dram, pers, wpool):
            p.release()

    nc.compile()
    return nc


def make_in_maps(inputs):
    f32 = lambda a: np.ascontiguousarray(np.asarray(a), dtype=np.float32)
    x = f32(inputs['x'])
    conv_w = f32(inputs['conv_w'])
    w = {
        'fc1_wT': np.ascontiguousarray(f32(inputs['fc1_w']).T),
        'fc1_b': f32(inputs['fc1_b']),
        'bn1_g': f32(inputs['bn1_g']),
        'bn1_b': f32(inputs['bn1_b']),
        'v1_T': np.ascontiguousarray((conv_w[:, :C] - conv_w[:, C:]).T),
        'cw2_T': np.ascontiguousarray(conv_w[:, C:].T),
        'conv_b': f32(inputs['conv_b']),
        'bnc_g': f32(inputs['bnc_g']),
        'bnc_b': f32(inputs['bnc_b']),
        'fc2_wT': np.ascontiguousarray(f32(inputs['fc2_w']).T),
        'fc2_b': f32(inputs['fc2_b']),
        'bn2_g': f32(inputs['bn2_g']),
        'bn2_b': f32(inputs['bn2_b']),
    }
    in_maps = []
    for d in range(8):
        b, qoff = d // 4, (d % 4) * NQ
        m = dict(w)
        m['x_own'] = np.ascontiguousarray(np.roll(x[b], -qoff, axis=0))
        in_maps.append(m)
    return in_maps


def kernel(**inputs):
    from concourse import bass_utils

    if 'nc' not in _CACHE:
        _CACHE['nc'] = _build()
    nc = _CACHE['nc']

    in_maps = make_in_maps(inputs)
    r = bass_utils.run_bass_kernel_spmd(nc, in_maps, core_ids=list(range(8)))
    _CACHE['last_res'] = r

    out = np.empty((B, N, C), np.float32)
    for d in range(8):
        b, qoff = d // 4, (d % 4) * NQ
        out[b, qoff:qoff + NQ] = r.results[d]['y']
    return out
